# revision 9
# baseline (speedup 1.0000x reference)
"""EncNet vq_codebook kernel for 8 Trainium2 NeuronCores.

Math (per reference):
  xs = x[:, :, 0, :].T                         # (b, s, c)
  d2[s,k]   = x2[s] - 2*cross[s,k] + cw2[k]
  a         = softmax_k(sm[k] * d2)
  e[b,k,c]  = sum_s a*xs - (sum_s a)*cw[k,c]
  BN over (b,c) (training stats), relu, mean over k, fc, sigmoid
  out = x * scale[b,c]

Distribution: data-parallel over batch (2 batches per core); BN batch
stats all-reduced across the 8 cores as a (64,2) tensor.

On-core layout trick: s-chunks of 128 land on SBUF partitions.  With an
x-chunk (c=128, s=128) as PE weights:
  - rhs = I                 -> xT chunk (s, c)       (transpose for free)
  - rhs = -2*sm_k*cw[k,c]   -> -2*sm_k*cross[s,k]
and with squared-x as weights:
  - rhs = sm[k] (replicated)-> sm_k * x2[s]
so PSUM directly holds L[s,k] = sm_k*(x2[s] - 2cross[s,k]).  The
constant exp(sm_k*cw2_k) factor is applied during the Z reduction
(tensor_tensor_reduce against a replicated-row constant), making
aw = exp(sm_k*d2) exactly, softmax without max-subtraction (logits
are <= ~0.006 by construction, so exp never overflows).
"""

import os
import sys

import numpy as np

try:
    import concourse.bass as bass  # noqa: F401
except ImportError:
    sys.path.insert(0, "/opt/trn_rl_repo")

import concourse.bacc as bacc
import concourse.bass as bass
import concourse.mybir as mybir
import concourse.tile as tile
from concourse.bass_utils import run_bass_kernel_spmd
from concourse._compat import get_trn_type
from ml_dtypes import bfloat16

F32 = mybir.dt.float32
BF16 = mybir.dt.bfloat16
ALU = mybir.AluOpType
ACTF = mybir.ActivationFunctionType

N_CORES = 8
B, C, SEQ, K = 16, 128, 16384, 64
B_LOC = B // N_CORES           # 2 batches per core
BIG = 2048                     # DMA chunk (free dim)
SUB = 128                      # s-subchunk = PSUM partition dim
BN_EPS = 1e-5


def build_program(seq=SEQ, b_loc=B_LOC, n_cores=N_CORES, big=BIG,
                  use_collective=True, use_gpsimd=True):
    n_big = seq // big
    n_sub = big // SUB

    nc = bacc.Bacc(
        get_trn_type() or "TRN2",
        target_bir_lowering=False,
        debug=False,
        num_devices=n_cores,
    )

    x_ap = nc.dram_tensor("x", [b_loc, C, seq], F32, kind="ExternalInput").ap()
    out_ap = nc.dram_tensor("out", [b_loc, C, seq], F32, kind="ExternalOutput").ap()

    def const_in(name, shape, dt):
        return nc.dram_tensor(name, shape, dt, kind="ExternalInput").ap()

    ident_d = const_in("ident_bf", [C, C], BF16)
    cwt_sm_d = const_in("cwt_sm_bf", [C, K], BF16)
    smrep_d = const_in("smrep_f32", [C, K], F32)
    wkrow_d = const_in("wkrow_f32", [SUB, K], F32)
    ones_d = const_in("ones_bf", [SUB, 1], BF16)
    cw_rows_d = const_in("cw_rows", [K, C], F32)
    gamma_d = const_in("gamma_col", [K, 1], F32)
    beta_d = const_in("beta_col", [K, 1], F32)
    fc_wt_d = const_in("fc_wt", [C, C], F32)
    fc_b_d = const_in("fc_b_col", [C, 1], F32)
    invk_d = const_in("invk_col", [K, 1], F32)

    with tile.TileContext(nc) as tc:
        with (
            tc.tile_pool(name="consts", bufs=1) as cpool,
            tc.tile_pool(name="xg", bufs=3) as xgp,
            tc.tile_pool(name="xsq", bufs=2) as xsqp,
            tc.tile_pool(name="xbf", bufs=2) as xbfp,
            tc.tile_pool(name="soft", bufs=4) as softp,
            tc.tile_pool(name="cols", bufs=6) as colp,
            tc.tile_pool(name="xt", bufs=3) as xtp,
            tc.tile_pool(name="etail", bufs=4) as etailp,
            tc.tile_pool(name="eloc", bufs=2) as elocp,
            tc.tile_pool(name="scales", bufs=2) as scalep,
            tc.tile_pool(name="og", bufs=3) as ogp,
            tc.tile_pool(name="ps_xt", bufs=2, space="PSUM") as ps_xt,
            tc.tile_pool(name="ps_L", bufs=2, space="PSUM") as ps_L,
            tc.tile_pool(name="ps_e", bufs=2, space="PSUM") as ps_e,
            tc.tile_pool(name="ps_tail", bufs=1, space="PSUM") as ps_tail,
            tc.tile_pool(name="dram", bufs=2, space="DRAM") as dram,
        ):
            # ---- load constants into SBUF once ----
            def load_const(dram_ap, shape, dt):
                t = cpool.tile(shape, dt, tag=dram_ap.tensor.name)
                nc.sync.dma_start(out=t[:], in_=dram_ap[:])
                return t

            ident = load_const(ident_d, [C, C], BF16)
            cwt_sm = load_const(cwt_sm_d, [C, K], BF16)
            smrep = load_const(smrep_d, [C, K], F32)
            wkrow = load_const(wkrow_d, [SUB, K], F32)
            ones_bf = load_const(ones_d, [SUB, 1], BF16)
            cw_rows = load_const(cw_rows_d, [K, C], F32)
            gamma = load_const(gamma_d, [K, 1], F32)
            beta = load_const(beta_d, [K, 1], F32)
            fc_wt = load_const(fc_wt_d, [C, C], F32)
            fc_b = load_const(fc_b_d, [C, 1], F32)
            invk = load_const(invk_d, [K, 1], F32)

            # ---- phase 1: per-batch aggregation e1|asum ----
            e_sbs = []
            for b in range(b_loc):
                e_ps = ps_e.tile([K, C + 1], F32)
                for j in range(n_big):
                    xg = xgp.tile([C, big], F32)
                    nc.sync.dma_start(
                        out=xg[:], in_=x_ap[b, :, j * big : (j + 1) * big]
                    )
                    xsq = xsqp.tile([C, big], F32)
                    xbf = xbfp.tile([C, big], BF16)
                    if use_gpsimd:
                        nc.gpsimd.tensor_tensor(xsq[:], xg[:], xg[:], ALU.mult)
                        nc.gpsimd.tensor_copy(xbf[:], xg[:])
                    else:
                        nc.vector.tensor_tensor(xsq[:], xg[:], xg[:], ALU.mult)
                        nc.vector.tensor_copy(xbf[:], xg[:])
                    for i in range(n_sub):
                        sl = slice(i * SUB, (i + 1) * SUB)
                        w_bf = xbf[:, sl]
                        w_sq = xsq[:, sl]
                        # xT chunk: (s,c) = x_chunk.T via identity rhs
                        xt_ps = ps_xt.tile([SUB, C], F32)
                        nc.tensor.matmul(
                            xt_ps[:], lhsT=w_bf, rhs=ident[:], start=True, stop=True
                        )
                        # L[s,k] = sm_k*(x2 - 2cross)
                        L_ps = ps_L.tile([SUB, K], F32)
                        nc.tensor.matmul(
                            L_ps[:], lhsT=w_bf, rhs=cwt_sm[:], start=True, stop=False
                        )
                        nc.tensor.matmul(
                            L_ps[:], lhsT=w_sq, rhs=smrep[:], start=False, stop=True
                        )
                        araw = softp.tile([SUB, K], BF16, tag="araw")
                        nc.scalar.activation(araw[:], L_ps[:], ACTF.Exp)
                        # aw = araw * exp(sm*cw2); Zw = sum_k aw
                        # (tensor_tensor_reduce hangs TRN2 hw here; split it)
                        aw = softp.tile([SUB, K], BF16, tag="aw")
                        nc.vector.tensor_tensor(aw[:], araw[:], wkrow[:], ALU.mult)
                        zw = colp.tile([SUB, 1], F32, tag="zw")
                        nc.vector.tensor_reduce(
                            zw[:], aw[:], mybir.AxisListType.X, ALU.add
                        )
                        rz = colp.tile([SUB, 1], F32, tag="rz")
                        nc.vector.reciprocal(rz[:], zw[:])
                        an = softp.tile([SUB, K], BF16, tag="an")
                        nc.vector.tensor_scalar(
                            out=an[:], in0=aw[:], scalar1=rz[:], scalar2=None,
                            op0=ALU.mult,
                        )
                        xt_sb = xtp.tile([SUB, C], BF16)
                        nc.vector.tensor_copy(xt_sb[:], xt_ps[:])
                        first = j == 0 and i == 0
                        last = j == n_big - 1 and i == n_sub - 1
                        nc.tensor.matmul(
                            e_ps[:, 0:C], lhsT=an[:], rhs=xt_sb[:],
                            start=first, stop=last, skip_group_check=True,
                        )
                        # NB: start=False even on the first chunk — MM above
                        # already marked this PSUM zero region; first touch of
                        # still-pending bytes overwrites, later ones accumulate.
                        nc.tensor.matmul(
                            e_ps[:, C : C + 1], lhsT=an[:], rhs=ones_bf[:],
                            start=False, stop=last, skip_group_check=True,
                        )
                e_sb = etailp.tile([K, C + 1], F32, tag="e_sb")
                nc.vector.tensor_copy(e_sb[:], e_ps[:])
                e_sbs.append(e_sb)

            # ---- local e + stats ----
            s1s, s2s, e_locs = [], [], []
            for b in range(b_loc):
                e_sb = e_sbs[b]
                easm = etailp.tile([K, C], F32, tag="easm")
                nc.vector.tensor_scalar(
                    out=easm[:], in0=cw_rows[:], scalar1=e_sb[:, C : C + 1],
                    scalar2=None, op0=ALU.mult,
                )
                e_loc = elocp.tile([K, C], F32)
                nc.vector.tensor_tensor(e_loc[:], e_sb[:, 0:C], easm[:], ALU.subtract)
                e_locs.append(e_loc)
                s1 = colp.tile([K, 1], F32, tag="s1")
                nc.vector.tensor_reduce(s1[:], e_loc[:], mybir.AxisListType.X, ALU.add)
                esq = etailp.tile([K, C], F32, tag="esq")
                nc.vector.tensor_tensor(esq[:], e_loc[:], e_loc[:], ALU.mult)
                s2 = colp.tile([K, 1], F32, tag="s2")
                nc.vector.tensor_reduce(s2[:], esq[:], mybir.AxisListType.X, ALU.add)
                s1s.append(s1)
                s2s.append(s2)

            stats = etailp.tile([K, 2], F32, tag="stats")
            nc.vector.tensor_tensor(stats[:, 0:1], s1s[0][:], s1s[1][:], ALU.add)
            nc.vector.tensor_tensor(stats[:, 1:2], s2s[0][:], s2s[1][:], ALU.add)

            # ---- all-reduce BN stats across cores ----
            cc_in = dram.tile([K, 2], F32)
            cc_out = dram.tile([K, 2], F32)
            nc.sync.dma_start(out=cc_in[:], in_=stats[:])
            if use_collective:
                nc.gpsimd.collective_compute(
                    "AllReduce",
                    ALU.add,
                    replica_groups=[list(range(n_cores))],
                    ins=[cc_in.opt()],
                    outs=[cc_out.opt()],
                )
            else:
                # debug: local stats only (wrong numerics, 1-core scale)
                nc.sync.dma_start(out=cc_out[:], in_=cc_in[:])
            gst = etailp.tile([K, 2], F32, tag="gst")
            nc.sync.dma_start(out=gst[:], in_=cc_out[:])

            # ---- BN affine + relu + mean_k + fc + sigmoid (tiny) ----
            n_tot = float(B * C)  # stats population: all b, all c
            mean = colp.tile([K, 1], F32, tag="mean")
            nc.vector.tensor_scalar(
                out=mean[:], in0=gst[:, 0:1], scalar1=1.0 / n_tot, scalar2=None,
                op0=ALU.mult,
            )
            ex2 = colp.tile([K, 1], F32, tag="ex2")
            nc.vector.tensor_scalar(
                out=ex2[:], in0=gst[:, 1:2], scalar1=1.0 / n_tot, scalar2=None,
                op0=ALU.mult,
            )
            msq = colp.tile([K, 1], F32, tag="msq")
            nc.vector.tensor_tensor(msq[:], mean[:], mean[:], ALU.mult)
            varep = colp.tile([K, 1], F32, tag="varep")
            nc.vector.tensor_tensor(varep[:], ex2[:], msq[:], ALU.subtract)
            nc.vector.tensor_scalar(
                out=varep[:], in0=varep[:], scalar1=BN_EPS, scalar2=None, op0=ALU.add
            )
            stdv = colp.tile([K, 1], F32, tag="stdv")
            nc.scalar.sqrt(stdv[:], varep[:])
            rstd = colp.tile([K, 1], F32, tag="rstd")
            nc.vector.reciprocal(rstd[:], stdv[:])
            psc = colp.tile([K, 1], F32, tag="psc")
            nc.vector.tensor_tensor(psc[:], gamma[:], rstd[:], ALU.mult)
            mps = colp.tile([K, 1], F32, tag="mps")
            nc.vector.tensor_tensor(mps[:], mean[:], psc[:], ALU.mult)
            pofs = colp.tile([K, 1], F32, tag="pofs")
            nc.vector.tensor_tensor(pofs[:], beta[:], mps[:], ALU.subtract)

            scale_cols = []
            for b in range(b_loc):
                reb = etailp.tile([K, C], F32, tag="reb")
                nc.scalar.activation(
                    reb[:], e_locs[b][:], ACTF.Relu, bias=pofs[:], scale=psc[:]
                )
                en_ps = ps_tail.tile([C, 1], F32, tag="tail")
                nc.tensor.matmul(
                    en_ps[:], lhsT=reb[:], rhs=invk[:], start=True, stop=True
                )
                en_sb = colp.tile([C, 1], F32, tag="en_sb")
                nc.vector.tensor_copy(en_sb[:], en_ps[:])
                fc_ps = ps_tail.tile([C, 1], F32, tag="tail")
                nc.tensor.matmul(
                    fc_ps[:], lhsT=fc_wt[:], rhs=en_sb[:], start=True, stop=True
                )
                sc = scalep.tile([C, 1], F32)
                nc.scalar.activation(sc[:], fc_ps[:], ACTF.Sigmoid, bias=fc_b[:])
                scale_cols.append(sc)

            # ---- phase 2: out = x * scale ----
            for b in range(b_loc):
                for j in range(n_big):
                    xg2 = xgp.tile([C, big], F32)
                    nc.sync.dma_start(
                        out=xg2[:], in_=x_ap[b, :, j * big : (j + 1) * big]
                    )
                    og = ogp.tile([C, big], F32)
                    nc.vector.tensor_scalar(
                        out=og[:], in0=xg2[:], scalar1=scale_cols[b][:],
                        scalar2=None, op0=ALU.mult,
                    )
                    nc.sync.dma_start(
                        out=out_ap[b, :, j * big : (j + 1) * big], in_=og[:]
                    )

    nc.compile()
    return nc


def make_const_inputs(codewords, smoothing, bn_weight, bn_bias, fc_w, fc_b):
    cw = np.asarray(codewords, np.float32)        # (K, C)
    sm = np.asarray(smoothing, np.float32)        # (K,)
    cw2 = (cw * cw).sum(1)                        # (K,)
    consts = {
        "ident_bf": np.eye(C, dtype=bfloat16),
        "cwt_sm_bf": (cw.T * (-2.0 * sm)[None, :]).astype(bfloat16),  # (C,K)
        "smrep_f32": np.tile(sm[None, :], (C, 1)).astype(np.float32),
        "wkrow_f32": np.tile(np.exp(sm * cw2)[None, :], (SUB, 1)).astype(np.float32),
        "ones_bf": np.ones((SUB, 1), dtype=bfloat16),
        "cw_rows": np.ascontiguousarray(cw),
        "gamma_col": np.asarray(bn_weight, np.float32).reshape(K, 1),
        "beta_col": np.asarray(bn_bias, np.float32).reshape(K, 1),
        "fc_wt": np.ascontiguousarray(np.asarray(fc_w, np.float32).T),  # (C_in,C_out)
        "fc_b_col": np.asarray(fc_b, np.float32).reshape(C, 1),
        "invk_col": np.full((K, 1), 1.0 / K, np.float32),
    }
    return consts


_NC_CACHE = {}


def _get_program():
    key = (SEQ, B_LOC, N_CORES, BIG)
    if key not in _NC_CACHE:
        _NC_CACHE[key] = build_program(*key)
    return _NC_CACHE[key]


def _run(inputs, trace=False, trace_kwargs=None):
    x = np.asarray(inputs["x"], np.float32)
    assert x.shape == (B, C, 1, SEQ), x.shape
    xs = np.ascontiguousarray(x.reshape(B, C, SEQ))
    consts = make_const_inputs(
        inputs["codewords"], inputs["smoothing"], inputs["bn_weight"],
        inputs["bn_bias"], inputs["fc_w"], inputs["fc_b"],
    )
    in_maps = [
        {"x": np.ascontiguousarray(xs[i * B_LOC : (i + 1) * B_LOC]), **consts}
        for i in range(N_CORES)
    ]
    nc = _get_program()
    res = run_bass_kernel_spmd(
        nc, in_maps, core_ids=list(range(N_CORES)), trace=trace,
        **(trace_kwargs or {}),
    )
    out = np.concatenate([res.results[i]["out"] for i in range(N_CORES)], axis=0)
    return out.reshape(B, C, 1, SEQ).astype(np.float32), res


def kernel(**inputs):
    out, _ = _run(inputs)
    return out
